# revision 2
# baseline (speedup 1.0000x reference)
"""Continuous-time LSTM Trainium2 kernel.

Strategy: 8-way data-parallel over batch (16 samples/core). Per core, the
L=512-step recurrence runs fully unrolled. The gate GEMM h @ W_h.T runs in
bf16 on the PE (4 K-chunks x 28 M-chunks, N=16). The x-projection
x @ W_x.T + bias is computed just-in-time every CH steps as wide (N=CH*16)
bf16 matmuls into SBUF, so the per-step GEMM only contracts over D=512.

All transcendentals come from the single ACT table set `exp_and_others`
(exp, tanh, abs, relu, square, identity): sigmoid is 0.5+0.5*tanh(x/2) and
softplus is relu(x) + poly(exp(-|x|)) with a degree-5 fitted polynomial for
log1p on (0,1]. This avoids any per-step ACT table reloads.

State layout everywhere: (128 partitions = d within 128-chunk,
free col = dchunk*16 + b), so the next step's GEMM consumes h directly as
the moving operand with zero transposes.
"""
import sys, os
for _p in ("/opt/trn_rl_repo", "/root/.axon_site/_ro/trn_rl_repo"):
    if os.path.isdir(_p) and _p not in sys.path:
        sys.path.insert(0, _p)

import numpy as np
from concourse import bacc, mybir, tile
import concourse.bass as bass
from concourse.bass_utils import run_bass_kernel_spmd

F32 = mybir.dt.float32
BF16 = mybir.dt.bfloat16
AF = mybir.ActivationFunctionType
OP = mybir.AluOpType

B, L, I, D = 128, 512, 256, 512
NCORES = 8
BL = B // NCORES          # 16 batch per core
W4 = 4 * BL               # 64: free width of one (dchunk, b) tile group
CH = 32                   # JIT x-projection chunk length (steps)
TD0 = 255                 # first step whose states are dumped (seq_lens >= 256)

# --- degree-5 fit of log1p(w) on [0,1] with the linear coeff pinned to 1 ---
_w = np.linspace(0.0, 1.0, 4001)
_target = np.log1p(_w) - _w
_A = np.stack([_w**k for k in range(2, 6)], axis=1)
_coef, *_ = np.linalg.lstsq(_A, _target, rcond=None)
LOG1P_C = [float(c) for c in _coef]  # coeffs for w^2..w^5
_fit_err = np.abs(_w + _A @ _coef - np.log1p(_w)).max()
assert _fit_err < 5e-5, _fit_err


def _softplus(x):
    return np.maximum(x, 0.0) + np.log1p(np.exp(-np.abs(x)))


def _host_init_state(bos, weight, bias_v):
    """One reference LSTM step on the BOS token from zero state (fp64)."""
    w = weight.astype(np.float64)
    b = bias_v.astype(np.float64)
    h = np.zeros(D)
    proj = w @ np.concatenate([bos.astype(np.float64), h]) + b
    sig = 1.0 / (1.0 + np.exp(-proj[: 5 * D]))
    i_g, f_g, ie_g, fe_g, o_n = np.split(sig, 5)
    z = np.tanh(proj[5 * D : 6 * D])
    cs_n = i_g * z          # f_g * c with c = 0
    ce_n = ie_g * z         # fe_g * ce with ce = 0
    d_n = _softplus(proj[6 * D : 7 * D])
    return np.stack([o_n, cs_n, ce_n, d_n]).astype(np.float32)  # (4, D)


_PROG_CACHE = {}


def build_program(steps=L):
    if steps in _PROG_CACHE:
        return _PROG_CACHE[steps]
    nchunks = (steps + CH - 1) // CH
    nd = max(steps - TD0, 1) if steps > TD0 else 0

    nc = bacc.Bacc("TRN2", target_bir_lowering=False)
    whT_d = nc.dram_tensor("whT", (128, 4, 3584), BF16, kind="ExternalInput")
    wxT_d = nc.dram_tensor("wxT", (128, 2, 3584), BF16, kind="ExternalInput")
    xT_d = nc.dram_tensor("xT", (128, 2, nchunks * CH * BL), BF16, kind="ExternalInput")
    dtn_d = nc.dram_tensor("dtn", (128, nchunks * CH * W4), F32, kind="ExternalInput")
    bias_d = nc.dram_tensor("bias", (128, 28), F32, kind="ExternalInput")
    init_d = nc.dram_tensor("init", (128, 4 * W4), F32, kind="ExternalInput")
    hout_d = nc.dram_tensor("hout", (steps, 128, W4), F32, kind="ExternalOutput")
    if nd:
        sdump_d = nc.dram_tensor("sdump", (nd, 128, 2 * W4), F32, kind="ExternalOutput")
        odump_d = nc.dram_tensor("odump", (nd, 128, W4), F32, kind="ExternalOutput")
        pdump_d = nc.dram_tensor("pdump", (nd, 128, W4), F32, kind="ExternalOutput")

    with tile.TileContext(nc) as tc:
        with (
            tc.tile_pool(name="const", bufs=1) as cpool,
            tc.tile_pool(name="xin", bufs=2) as xpool,
            tc.tile_pool(name="px", bufs=2) as pxpool,
            tc.tile_pool(name="state", bufs=3) as spool,
            tc.tile_pool(name="work", bufs=3) as wpool,
            tc.tile_pool(name="gpsum", bufs=2, space="PSUM") as gpool,
            tc.tile_pool(name="pxpsum", bufs=2, space="PSUM") as ppool,
        ):
            whT = cpool.tile((128, 4, 3584), BF16)
            wxT = cpool.tile((128, 2, 3584), BF16)
            biasT = cpool.tile((128, 28), F32)
            initT = cpool.tile((128, 4 * W4), F32)
            nc.sync.dma_start(whT[:], whT_d[:])
            nc.sync.dma_start(wxT[:], wxT_d[:])
            nc.sync.dma_start(biasT[:], bias_d[:])
            nc.sync.dma_start(initT[:], init_d[:])

            o_p = initT[:, 0 * W4 : 1 * W4]
            cs_p = initT[:, 1 * W4 : 2 * W4]
            ce_p = initT[:, 2 * W4 : 3 * W4]
            d_p = initT[:, 3 * W4 : 4 * W4]

            pxc = xc = dtc = None
            for t in range(steps):
                chunk, tl = divmod(t, CH)
                if tl == 0:
                    ntok = CH * BL
                    xc = xpool.tile((128, 2, ntok), BF16, tag="xc")
                    nc.sync.dma_start(xc[:], xT_d[:, :, chunk * ntok : (chunk + 1) * ntok])
                    dtc = xpool.tile((128, CH * W4), F32, tag="dtc")
                    nc.sync.dma_start(dtc[:], dtn_d[:, chunk * CH * W4 : (chunk + 1) * CH * W4])
                    pxc = pxpool.tile((128, 28, ntok), BF16, tag="pxc")
                    for m in range(28):
                        pp = ppool.tile((128, ntok), F32, tag="pp")
                        nc.tensor.matmul(pp[:], wxT[:, 0, m * 128 : (m + 1) * 128], xc[:, 0, :], start=True, stop=False)
                        nc.tensor.matmul(pp[:], wxT[:, 1, m * 128 : (m + 1) * 128], xc[:, 1, :], start=False, stop=True)
                        nc.scalar.activation(pxc[:, m, :], pp[:], AF.Identity, bias=biasT[:, m : m + 1])

                # --- decay + h (critical path) ---
                md = wpool.tile((128, W4), F32, tag="md")
                nc.vector.tensor_mul(md[:], d_p, dtc[:, tl * W4 : (tl + 1) * W4])
                edt = wpool.tile((128, W4), F32, tag="edt")
                nc.scalar.activation(edt[:], md[:], AF.Exp)
                cdel = wpool.tile((128, W4), F32, tag="cdel")
                nc.vector.tensor_sub(cdel[:], cs_p, ce_p)
                cde = wpool.tile((128, W4), F32, tag="cde")
                nc.vector.tensor_mul(cde[:], cdel[:], edt[:])
                c = wpool.tile((128, W4), F32, tag="c")
                nc.vector.tensor_add(c[:], cde[:], ce_p)
                tanhc = wpool.tile((128, W4), F32, tag="tanhc")
                nc.scalar.activation(tanhc[:], c[:], AF.Tanh)
                h32 = wpool.tile((128, W4), F32, tag="h32")
                nc.vector.tensor_mul(h32[:], o_p, tanhc[:])
                hbf = wpool.tile((128, W4), BF16, tag="hbf")
                nc.vector.tensor_copy(hbf[:], h32[:])
                nc.sync.dma_start(hout_d[t], h32[:])

                # --- gate GEMM: G[m] += whT[k,m].T @ h[k] ---
                G = gpool.tile((128, 28, BL), F32, tag="G")
                for m in range(28):
                    for k in range(4):
                        nc.tensor.matmul(
                            G[:, m, :],
                            whT[:, k, m * 128 : (m + 1) * 128],
                            hbf[:, k * BL : (k + 1) * BL],
                            start=(k == 0),
                            stop=(k == 3),
                        )
                # add x-projection + bias (precomputed, bf16)
                nc.vector.tensor_add(G[:], G[:], pxc[:, :, tl * BL : (tl + 1) * BL])

                # --- gates ---
                sig = wpool.tile((128, 20, BL), F32, tag="sig")
                nc.scalar.activation(sig[:], G[:, 0:20, :], AF.Tanh, scale=0.5)
                siga = spool.tile((128, 20, BL), F32, tag="siga")
                nc.vector.tensor_scalar(siga[:], sig[:], 0.5, 0.5, OP.mult, OP.add)
                z = wpool.tile((128, W4), F32, tag="z")
                nc.scalar.activation(z[:], G[:, 20:24, :], AF.Tanh)

                # softplus(pd) = relu(pd) + log1p(exp(-|pd|))
                absd = wpool.tile((128, W4), F32, tag="absd")
                nc.scalar.activation(absd[:], G[:, 24:28, :], AF.Abs)
                w1 = wpool.tile((128, W4), F32, tag="w1")
                nc.scalar.activation(w1[:], absd[:], AF.Exp, scale=-1.0)
                w2 = wpool.tile((128, W4), F32, tag="w2")
                nc.scalar.activation(w2[:], w1[:], AF.Square)
                w3 = wpool.tile((128, W4), F32, tag="w3")
                nc.vector.tensor_mul(w3[:], w1[:], w2[:])
                w4 = wpool.tile((128, W4), F32, tag="w4")
                nc.scalar.activation(w4[:], w2[:], AF.Square)
                w5 = wpool.tile((128, W4), F32, tag="w5")
                nc.vector.tensor_mul(w5[:], w2[:], w3[:])
                ph = wpool.tile((128, W4), F32, tag="ph")
                nc.vector.scalar_tensor_tensor(ph[:], w2[:], LOG1P_C[0], w1[:], OP.mult, OP.add)
                ph2 = wpool.tile((128, W4), F32, tag="ph2")
                nc.vector.scalar_tensor_tensor(ph2[:], w3[:], LOG1P_C[1], ph[:], OP.mult, OP.add)
                ph3 = wpool.tile((128, W4), F32, tag="ph3")
                nc.vector.scalar_tensor_tensor(ph3[:], w4[:], LOG1P_C[2], ph2[:], OP.mult, OP.add)
                ph4 = wpool.tile((128, W4), F32, tag="ph4")
                nc.vector.scalar_tensor_tensor(ph4[:], w5[:], LOG1P_C[3], ph3[:], OP.mult, OP.add)
                relud = wpool.tile((128, W4), F32, tag="relud")
                nc.scalar.activation(relud[:], G[:, 24:28, :], AF.Relu)
                dn = spool.tile((128, W4), F32, tag="dn")
                nc.vector.tensor_add(dn[:], relud[:], ph4[:])

                # --- new states ---
                sn = spool.tile((128, 2 * W4), F32, tag="sn")
                fc = wpool.tile((128, W4), F32, tag="fc")
                nc.vector.tensor_mul(fc[:], siga[:, 4:8, :], c[:])
                iz = wpool.tile((128, W4), F32, tag="iz")
                nc.vector.tensor_mul(iz[:], siga[:, 0:4, :], z[:])
                nc.vector.tensor_add(sn[:, 0:W4], fc[:], iz[:])
                fece = wpool.tile((128, W4), F32, tag="fece")
                nc.vector.tensor_mul(fece[:], siga[:, 12:16, :], ce_p)
                iez = wpool.tile((128, W4), F32, tag="iez")
                nc.vector.tensor_mul(iez[:], siga[:, 8:12, :], z[:])
                nc.vector.tensor_add(sn[:, W4 : 2 * W4], fece[:], iez[:])

                if steps > TD0 and t >= TD0:
                    nc.sync.dma_start(sdump_d[t - TD0], sn[:])
                    nc.sync.dma_start(odump_d[t - TD0], siga[:, 16:20, :])
                    pd = wpool.tile((128, W4), F32, tag="pd")
                    nc.scalar.activation(pd[:], G[:, 24:28, :], AF.Identity)
                    nc.sync.dma_start(pdump_d[t - TD0], pd[:])

                o_p = siga[:, 16:20, :]
                cs_p = sn[:, 0:W4]
                ce_p = sn[:, W4 : 2 * W4]
                d_p = dn[:]

    nc.compile()
    _PROG_CACHE[steps] = nc
    return nc


def prep_core_inputs(x, time_deltas, seq_lens, bos, weight, bias, steps=L):
    bias_v = bias
    """Host-side shard + relayout. Returns (in_maps, init_4D)."""
    nchunks = (steps + CH - 1) // CH
    lpad = nchunks * CH
    mask = (np.arange(steps)[None, :] < seq_lens[:, None]).astype(np.float32)
    xm = x[:, :steps] * mask[:, :, None]
    dtm = time_deltas[:, :steps] * mask

    init4 = _host_init_state(bos, weight, bias_v)  # (4, D)
    whT = np.ascontiguousarray(weight[:, I:].T).astype(np.float32)  # (512, 3584)
    wxT = np.ascontiguousarray(weight[:, :I].T).astype(np.float32)  # (256, 3584)
    whT_t = whT.reshape(4, 128, 3584).transpose(1, 0, 2)  # (128,4,3584)
    wxT_t = wxT.reshape(2, 128, 3584).transpose(1, 0, 2)
    whT_bf = _to_bf16(whT_t)
    wxT_bf = _to_bf16(wxT_t)
    bias_t = np.ascontiguousarray(bias_v.reshape(28, 128).T)  # (128, 28)

    init_t = np.zeros((128, 4 * W4), np.float32)
    for s in range(4):
        for dc in range(4):
            init_t[:, s * W4 + dc * BL : s * W4 + (dc + 1) * BL] = init4[s, dc * 128 : (dc + 1) * 128][:, None]

    in_maps = []
    for cidx in range(NCORES):
        bs = slice(cidx * BL, (cidx + 1) * BL)
        xs = xm[bs]  # (BL, steps, I)
        dts = dtm[bs]  # (BL, steps)
        if lpad > steps:
            xs = np.concatenate([xs, np.zeros((BL, lpad - steps, I), np.float32)], axis=1)
            dts = np.concatenate([dts, np.zeros((BL, lpad - steps), np.float32)], axis=1)
        xT = xs.transpose(2, 1, 0).reshape(I, lpad * BL)  # [i, l*BL+b]
        xT = xT.reshape(2, 128, lpad * BL).transpose(1, 0, 2)  # (128,2,tok)
        dtn = np.ascontiguousarray(
            np.broadcast_to(-dts.T[:, None, :], (lpad, 4, BL)).reshape(1, lpad * W4)
        ) * np.ones((128, 1), np.float32)
        in_maps.append({
            "whT": whT_bf,
            "wxT": wxT_bf,
            "xT": _to_bf16(xT),
            "dtn": dtn.astype(np.float32),
            "bias": bias_t,
            "init": init_t,
        })
    return in_maps, init4


def _to_bf16(a):
    import jax.numpy as jnp
    return np.asarray(jnp.asarray(a, dtype=jnp.bfloat16))


def unshard(results, seq_lens, steps=L):
    """Reassemble full outputs (B, steps, D) and final (1, B, 4D)."""
    outputs = np.empty((B, steps, D), np.float32)
    final = np.empty((B, 4 * D), np.float32)
    for cidx in range(NCORES):
        r = results[cidx]
        hout = r["hout"]  # (steps, 128, W4)
        for dc in range(4):
            # outputs[b, l, dc*128+p] = hout[l, p, dc*16+b]
            blk = hout[:, :, dc * BL : (dc + 1) * BL]  # (steps,128,BL)
            outputs[cidx * BL : (cidx + 1) * BL, :, dc * 128 : (dc + 1) * 128] = blk.transpose(2, 0, 1)
        if steps > TD0:
            sd, od, pdp = r["sdump"], r["odump"], r["pdump"]
            for b in range(BL):
                tstar = int(np.clip(seq_lens[cidx * BL + b] - 1, 0, steps - 1)) - TD0
                assert tstar >= 0
                row = np.empty(4 * D, np.float32)
                for dc in range(4):
                    col = dc * BL + b
                    row[0 * D + dc * 128 : 0 * D + (dc + 1) * 128] = od[tstar, :, col]
                    row[1 * D + dc * 128 : 1 * D + (dc + 1) * 128] = sd[tstar, :, col]
                    row[2 * D + dc * 128 : 2 * D + (dc + 1) * 128] = sd[tstar, :, W4 + col]
                    row[3 * D + dc * 128 : 3 * D + (dc + 1) * 128] = _softplus(pdp[tstar, :, col])
                final[cidx * BL + b] = row
    return outputs, final[None]


def kernel(x, time_deltas, seq_lens, bos, weight, bias, **_):
    nc = build_program(L)
    in_maps, _ = prep_core_inputs(x, time_deltas, seq_lens, bos, weight, bias, L)
    res = run_bass_kernel_spmd(nc, in_maps, core_ids=list(range(NCORES)))
    return unshard(res.results, seq_lens, L)
